# revision 8
# baseline (speedup 1.0000x reference)
"""FBGCN layer on 8 Trainium2 NeuronCores.

Math (reference):
    Lhp = d_inv @ lap @ d_inv
    Hh  = Lhp @ relu(x @ W_high)
    Hl  = relu(gcn_conv(x, edge_index, W_conv, b_conv))
    out = aL * Hl + aH * Hh

Kernel strategy:
  * Re-associate the high-pass chain: Hh = d_inv @ (lap @ (d_inv @ u)),
    u = relu(x @ W_high) — three thin [N,N]@[N,256] matmuls instead of two
    N^3 matmuls (10x fewer FLOPs).
  * GCN scatter-add becomes a dense matmul agg = A @ xw where
    A[dst,src] = sum of edge norms (built on host from the indices; this is
    index preprocessing, all feature-payload compute stays on device).
  * aL is folded into A and b_conv (aL*relu(y) == relu(aL*y) for aL>=0);
    aH is folded into lap. No runtime scalars reach the device.
  * 1D row-shard over 8 cores; AllGather the thin intermediates (u, xw, v, w)
    between stages; big matrices are consumed as column-slices per core.
  * Matmuls run as float32r (full PE rate) via AP bitcast; fp32 storage.
"""

import sys
import types
from contextlib import ExitStack

import numpy as np

N = 8192
E = 262144
D = 256
N_CORES = 8
RPC = N // N_CORES  # rows per core

LAST_EXEC_NS = None
LAST_RESULTS = None

_PROGRAM_CACHE = {}


def _patch_tile_drain(tile, mybir):
    """Split the Tile exit-drain's sem waits across multiple Drain
    instructions: this walrus build rejects >1 sync wait on a Drain op
    ("Too many sync wait commands")."""
    if getattr(tile.TileContext, "_drain_patched", False):
        return
    from concourse.vector_clock import ScopedClock

    def _patched(self, tick_clock, wait_clock):
        drain_inst = self.nc.sync.drain()
        wait_clock.add_sem_waits(
            drain_inst.ins, ScopedClock({None: tick_clock.global_clock})
        )
        si = drain_inst.ins.sync_info
        if si is not None and si.on_wait and len(si.on_wait) > 1:
            waits = list(si.on_wait)
            si.on_wait = waits[:1]
            for w in waits[1:]:
                extra = self.nc.sync.drain()
                extra.ins.sync_info = mybir.SyncInfo(on_wait=[w], on_update=[])
        self.nc.all_engine_barrier()
        popped = self.nc._tile_sem_poison_stack.pop()
        assert popped is self._sem_poison
        self.nc.clear_and_free_semaphores(list(self.sems.allocated().values()))
        self.nc.all_engine_barrier()

    tile.TileContext._drain_and_barrier = _patched
    tile.TileContext._drain_patched = True


def _split_excess_waits(nc, mybir, max_waits=1):
    """This walrus build rejects instructions carrying more than one sync
    wait ("Too many sync wait commands"). Hoist excess waits onto
    EventSemaphore carriers inserted just before the instruction on the
    same engine — serial execution makes this equivalent."""
    counter = [0]
    for fn in nc.m.functions:
        for blk in fn.blocks:
            insts = list(blk.instructions)
            if not any(
                i.sync_info is not None
                and i.sync_info.on_wait
                and len(i.sync_info.on_wait) > max_waits
                for i in insts
            ):
                continue
            new = []
            for inst in insts:
                si = inst.sync_info
                if si is not None and si.on_wait and len(si.on_wait) > max_waits:
                    waits = list(si.on_wait)
                    for w in waits[:-max_waits]:
                        counter[0] += 1
                        carrier = mybir.InstEventSemaphore(
                            name=f"wait_split_{counter[0]}", ins=[], outs=[]
                        )
                        carrier.engine = inst.engine
                        carrier.sync_info = mybir.SyncInfo(
                            on_wait=[w], on_update=[]
                        )
                        new.append(carrier)
                    si.on_wait = waits[-max_waits:]
                new.append(inst)
            blk.instructions = new
    return counter[0]


def _install_ntff_hook():
    """Register the axon NTFF profiling hook (the image's antenv package
    lacks axon_hooks; provide it so trace=True works)."""
    if "antenv.axon_hooks" in sys.modules:
        return
    try:
        import antenv
    except ImportError:
        antenv = types.ModuleType("antenv")
        sys.modules["antenv"] = antenv
    mod = types.ModuleType("antenv.axon_hooks")
    _state = {"h": None}
    mod.set_axon_ntff_profile_hook = lambda h: _state.__setitem__("h", h)
    mod.get_axon_ntff_profile_hook = lambda: _state["h"]
    sys.modules["antenv.axon_hooks"] = mod
    antenv.axon_hooks = mod
    try:
        from trn_agent_boot.trn_boot import _ntff_profile_via_ctypes

        mod.set_axon_ntff_profile_hook(
            _ntff_profile_via_ctypes("/opt/axon/libaxon_pjrt.so")
        )
    except Exception:
        pass


def _build_program():
    import concourse.bass as bass
    import concourse.mybir as mybir
    import concourse.tile as tile
    from concourse.kernels.tile_matmul import matmul_tile_kernel

    _patch_tile_drain(tile, mybir)

    f32 = mybir.dt.float32
    f32r = mybir.dt.float32r
    groups = [list(range(N_CORES))]

    nc = bass.Bass("TRN2", target_bir_lowering=False, num_devices=N_CORES)

    xT = nc.dram_tensor("xT", [D, N], f32, kind="ExternalInput")
    Wh = nc.dram_tensor("Wh", [D, D], f32, kind="ExternalInput")
    Wc = nc.dram_tensor("Wc", [D, D], f32, kind="ExternalInput")
    bp = nc.dram_tensor("bp", [D, 1], f32, kind="ExternalInput")
    dT = nc.dram_tensor("dT", [N, RPC], f32, kind="ExternalInput")
    lapT = nc.dram_tensor("lapT", [N, RPC], f32, kind="ExternalInput")
    AT = nc.dram_tensor("AT", [N, RPC], f32, kind="ExternalInput")
    outT = nc.dram_tensor("outT", [D, RPC], f32, kind="ExternalOutput")

    def r(ap):
        return ap.bitcast(f32r)

    with tile.TileContext(nc) as tc:
        with ExitStack() as stk:
            dram = stk.enter_context(tc.tile_pool(name="dram", bufs=1, space="DRAM"))
            v_c = dram.tile([RPC, D], f32)
            w_c = dram.tile([RPC, D], f32)
            u = dram.tile([N, D], f32)
            xw = dram.tile([N, D], f32)
            v = dram.tile([N, D], f32, addr_space="Shared")
            w = dram.tile([N, D], f32, addr_space="Shared")
            hhT = dram.tile([D, RPC], f32)
            aggT = dram.tile([D, RPC], f32)

            def allgather(src, dst):
                nc.gpsimd.collective_compute(
                    "AllGather",
                    mybir.AluOpType.bypass,
                    replica_groups=groups,
                    ins=[src.opt()],
                    outs=[dst.opt()],
                )

            # Full u = relu(x @ W_high), xw = x @ W_conv on every core
            # (replicated — cheaper than an AllGather barrier).
            matmul_tile_kernel(tc, r(xT[:]), r(Wh[:]), u[:], use_relu=True)
            matmul_tile_kernel(tc, r(xT[:]), r(Wc[:]), xw[:])

            with (
                tc.tile_pool(name="big_stream", bufs=4) as pool_big,
                tc.tile_pool(name="thin_cache", bufs=17) as pool_cache,
            ):
                # aggT = (aL*A[rows_c] @ xw).T = xw.T @ AT   [256, RPC]
                # Traced early so its DMA/PE work fills the AllGather
                # bubbles of the chain below (it has no deps on v/w).
                with (
                    tc.tile_pool(name="thin_stream_f", bufs=4) as pool_thin_f,
                    tc.tile_pool(name="big_stream_f", bufs=4) as pool_big_f,
                ):
                    matmul_tile_kernel(
                        tc, r(xw[:]), r(AT[:]), aggT[:], cache_tiles=False,
                        kxm_pool=pool_thin_f, kxn_pool=pool_big_f,
                    )

                # v_c = d_inv[rows_c] @ u  (kxm = dT cols streamed,
                # kxn = u cached in SBUF across both m-tiles)
                matmul_tile_kernel(
                    tc, r(dT[:]), r(u[:]), v_c[:],
                    kxm_pool=pool_big, kxn_pool=pool_cache,
                )
                allgather(v_c, v)

                # w_c = (aH * lap)[rows_c] @ v
                matmul_tile_kernel(
                    tc, r(lapT[:]), r(v[:]), w_c[:],
                    kxm_pool=pool_big, kxn_pool=pool_cache,
                )
                allgather(w_c, w)

            with (
                tc.tile_pool(name="thin_stream_e", bufs=4) as pool_thin_e,
                tc.tile_pool(name="big_stream_e", bufs=4) as pool_big_e,
            ):
                # hhT = (d_inv[rows_c] @ w).T = w.T @ dT   [256, RPC]
                matmul_tile_kernel(
                    tc, r(w[:]), r(dT[:]), hhT[:], cache_tiles=False,
                    kxm_pool=pool_thin_e, kxn_pool=pool_big_e,
                )

            # outT = relu(aggT + b') + hhT
            with tc.tile_pool(name="combine", bufs=2) as gp:
                for h in range(2):
                    rows = slice(h * 128, (h + 1) * 128)
                    at_t = gp.tile([128, RPC], f32, name="at")
                    hh_t = gp.tile([128, RPC], f32, name="hh")
                    b_t = gp.tile([128, 1], f32, name="bt")
                    nc.sync.dma_start(at_t[:], aggT[rows, :])
                    nc.sync.dma_start(hh_t[:], hhT[rows, :])
                    nc.sync.dma_start(b_t[:], bp[rows, :])
                    nc.scalar.activation(
                        at_t[:],
                        at_t[:],
                        mybir.ActivationFunctionType.Relu,
                        bias=b_t[:],
                    )
                    nc.vector.tensor_add(at_t[:], at_t[:], hh_t[:])
                    nc.sync.dma_start(outT[rows, :], at_t[:])

    _split_excess_waits(nc, mybir)
    return nc


def _get_program():
    if "nc" not in _PROGRAM_CACHE:
        _PROGRAM_CACHE["nc"] = _build_program()
    return _PROGRAM_CACHE["nc"]


def _host_prep(x, edge_index, lap, d_inv, W_high, W_conv, b_conv, aL, aH):
    x = np.asarray(x, dtype=np.float32)
    edge_index = np.asarray(edge_index)
    lap = np.asarray(lap, dtype=np.float32)
    d_inv = np.asarray(d_inv, dtype=np.float32)
    W_high = np.ascontiguousarray(np.asarray(W_high, dtype=np.float32))
    W_conv = np.ascontiguousarray(np.asarray(W_conv, dtype=np.float32))
    b_conv = np.asarray(b_conv, dtype=np.float32)
    aL = float(np.asarray(aL).reshape(-1)[0])
    aH = float(np.asarray(aH).reshape(-1)[0])

    n = x.shape[0]
    src = edge_index[0].astype(np.int64)
    dst = edge_index[1].astype(np.int64)
    loops = np.arange(n, dtype=np.int64)
    src_all = np.concatenate([src, loops])
    dst_all = np.concatenate([dst, loops])

    deg = np.bincount(dst_all, minlength=n).astype(np.float32)
    dis = np.where(deg > 0, 1.0 / np.sqrt(np.maximum(deg, 1.0)), 0.0).astype(
        np.float32
    )
    # aL folded into the adjacency (aL*relu(y) == relu(aL*y), aL >= 0)
    norm_all = (dis[src_all] * dis[dst_all] * np.float32(aL)).astype(np.float32)

    AT_full = np.zeros((n, n), dtype=np.float32)  # AT[src, dst]
    np.add.at(AT_full, (src_all, dst_all), norm_all)

    bprime = (np.float32(aL) * b_conv).reshape(D, 1).astype(np.float32)
    xT_full = np.ascontiguousarray(x.T)  # [256, N]

    in_maps = []
    for c in range(N_CORES):
        rows = slice(c * RPC, (c + 1) * RPC)
        in_maps.append(
            {
                "xT": xT_full,
                "Wh": W_high,
                "Wc": W_conv,
                "bp": bprime,
                "dT": np.ascontiguousarray(d_inv[rows, :].T),
                "lapT": np.ascontiguousarray(lap[rows, :].T) * np.float32(aH),
                "AT": np.ascontiguousarray(AT_full[:, rows]),
            }
        )
    return in_maps


def kernel(
    x,
    edge_index,
    lap,
    d_inv,
    W_high,
    W_conv,
    b_conv,
    aL,
    aH,
    _profile=False,
):
    global LAST_EXEC_NS, LAST_RESULTS
    from concourse.bass_utils import run_bass_kernel_spmd

    if _profile:
        _install_ntff_hook()

    in_maps = _host_prep(
        x, edge_index, lap, d_inv, W_high, W_conv, b_conv, aL, aH
    )
    nc = _get_program()
    res = run_bass_kernel_spmd(
        nc, in_maps, list(range(N_CORES)), trace=bool(_profile)
    )
    LAST_EXEC_NS = res.exec_time_ns
    LAST_RESULTS = res
    out = np.concatenate(
        [res.results[c]["outT"].T for c in range(N_CORES)], axis=0
    )
    return np.ascontiguousarray(out.astype(np.float32))


# revision 9
# speedup vs baseline: 1.7895x; 1.7895x over previous
"""FBGCN layer on 8 Trainium2 NeuronCores.

Math (reference):
    Lhp = d_inv @ lap @ d_inv
    Hh  = Lhp @ relu(x @ W_high)
    Hl  = relu(gcn_conv(x, edge_index, W_conv, b_conv))
    out = aL * Hl + aH * Hh

Kernel strategy:
  * Re-associate the high-pass chain: Hh = d_inv @ (lap @ (d_inv @ u)),
    u = relu(x @ W_high) — three thin [N,N]@[N,256] matmuls instead of two
    N^3 matmuls (10x fewer FLOPs).
  * GCN scatter-add becomes a dense matmul agg = A @ xw where
    A[dst,src] = sum of edge norms (built on host from the indices; this is
    index preprocessing, the feature-payload compute stays on device).
  * aL is folded into A and b_conv (aL*relu(y) == relu(aL*y) for aL>=0);
    aH is folded into lap. No runtime scalars reach the device.
  * 1D row-shard over 8 cores; AllGather the thin intermediates (v, w)
    between stages; u/xw computed replicated (cheaper than a collective).
  * Matmul operands are fp16 (full PE rate, half the HBM traffic;
    fp32 PSUM accumulation). Big matrices are host-pre-tiled into
    [m_block, 128, 64, 512] so every kxm/kxn DMA is contiguous.
"""

import sys
import types
from contextlib import ExitStack

import numpy as np

N = 8192
E = 262144
D = 256
N_CORES = 8
RPC = N // N_CORES  # rows per core
MB = 512            # m-block width of the pre-tiled big matrices

LAST_EXEC_NS = None
LAST_RESULTS = None

_PROGRAM_CACHE = {}


def _patch_tile_drain(tile, mybir):
    """Split the Tile exit-drain's sem waits across multiple Drain
    instructions: this walrus build rejects >1 sync wait on a Drain op
    ("Too many sync wait commands")."""
    if getattr(tile.TileContext, "_drain_patched", False):
        return
    from concourse.vector_clock import ScopedClock

    def _patched(self, tick_clock, wait_clock):
        drain_inst = self.nc.sync.drain()
        wait_clock.add_sem_waits(
            drain_inst.ins, ScopedClock({None: tick_clock.global_clock})
        )
        si = drain_inst.ins.sync_info
        if si is not None and si.on_wait and len(si.on_wait) > 1:
            waits = list(si.on_wait)
            si.on_wait = waits[:1]
            for w in waits[1:]:
                extra = self.nc.sync.drain()
                extra.ins.sync_info = mybir.SyncInfo(on_wait=[w], on_update=[])
        self.nc.all_engine_barrier()
        popped = self.nc._tile_sem_poison_stack.pop()
        assert popped is self._sem_poison
        self.nc.clear_and_free_semaphores(list(self.sems.allocated().values()))
        self.nc.all_engine_barrier()

    tile.TileContext._drain_and_barrier = _patched
    tile.TileContext._drain_patched = True


def _split_excess_waits(nc, mybir, max_waits=1):
    """This walrus build rejects instructions carrying more than one sync
    wait ("Too many sync wait commands"). Hoist excess waits onto
    EventSemaphore carriers inserted just before the instruction on the
    same engine — serial execution makes this equivalent."""
    counter = [0]
    for fn in nc.m.functions:
        for blk in fn.blocks:
            insts = list(blk.instructions)
            if not any(
                i.sync_info is not None
                and i.sync_info.on_wait
                and len(i.sync_info.on_wait) > max_waits
                for i in insts
            ):
                continue
            new = []
            for inst in insts:
                si = inst.sync_info
                if si is not None and si.on_wait and len(si.on_wait) > max_waits:
                    waits = list(si.on_wait)
                    for w in waits[:-max_waits]:
                        counter[0] += 1
                        carrier = mybir.InstEventSemaphore(
                            name=f"wait_split_{counter[0]}", ins=[], outs=[]
                        )
                        carrier.engine = inst.engine
                        carrier.sync_info = mybir.SyncInfo(
                            on_wait=[w], on_update=[]
                        )
                        new.append(carrier)
                    si.on_wait = waits[-max_waits:]
                new.append(inst)
            blk.instructions = new
    return counter[0]


def _install_ntff_hook():
    """Register the axon NTFF profiling hook (the image's antenv package
    lacks axon_hooks; provide it so trace=True works)."""
    if "antenv.axon_hooks" in sys.modules:
        return
    try:
        import antenv
    except ImportError:
        antenv = types.ModuleType("antenv")
        sys.modules["antenv"] = antenv
    mod = types.ModuleType("antenv.axon_hooks")
    _state = {"h": None}
    mod.set_axon_ntff_profile_hook = lambda h: _state.__setitem__("h", h)
    mod.get_axon_ntff_profile_hook = lambda: _state["h"]
    sys.modules["antenv.axon_hooks"] = mod
    antenv.axon_hooks = mod
    try:
        from trn_agent_boot.trn_boot import _ntff_profile_via_ctypes

        mod.set_axon_ntff_profile_hook(
            _ntff_profile_via_ctypes("/opt/axon/libaxon_pjrt.so")
        )
    except Exception:
        pass


def _build_program():
    import concourse.bass as bass
    import concourse.mybir as mybir
    import concourse.tile as tile
    from concourse.kernels.tile_matmul import matmul_tile_kernel

    _patch_tile_drain(tile, mybir)

    f32 = mybir.dt.float32
    f16 = mybir.dt.float16
    groups = [list(range(N_CORES))]
    NBLK = RPC // MB  # m-blocks per core

    nc = bass.Bass("TRN2", target_bir_lowering=False, num_devices=N_CORES)

    xT = nc.dram_tensor("xT", [D, N], f16, kind="ExternalInput")
    Wcat = nc.dram_tensor("Wcat", [D, 2 * D], f16, kind="ExternalInput")
    bp = nc.dram_tensor("bp", [D, 1], f32, kind="ExternalInput")
    # big matrices pre-tiled: [m_block, pi, po, mi]; k = po*128 + pi
    dT4 = nc.dram_tensor("dT4", [NBLK, 128, N // 128, MB], f16, kind="ExternalInput")
    lapT4 = nc.dram_tensor(
        "lapT4", [NBLK, 128, N // 128, MB], f16, kind="ExternalInput"
    )
    AT4 = nc.dram_tensor("AT4", [NBLK, 128, N // 128, MB], f16, kind="ExternalInput")
    outT = nc.dram_tensor("outT", [D, RPC], f32, kind="ExternalOutput")

    with tile.TileContext(nc) as tc:
        with ExitStack() as stk:
            dram = stk.enter_context(tc.tile_pool(name="dram", bufs=1, space="DRAM"))
            uxw = dram.tile([N, 2 * D], f16)  # [u | xw]
            v_c = dram.tile([RPC, D], f16)
            w_c = dram.tile([RPC, D], f16)
            v = dram.tile([N, D], f16, addr_space="Shared")
            w = dram.tile([N, D], f16, addr_space="Shared")
            hhT = dram.tile([D, RPC], f32)
            aggT = dram.tile([D, RPC], f32)
            u = uxw[:, 0:D]
            xw = uxw[:, D : 2 * D]

            def allgather(src, dst):
                nc.gpsimd.collective_compute(
                    "AllGather",
                    mybir.AluOpType.bypass,
                    replica_groups=groups,
                    ins=[src.opt()],
                    outs=[dst.opt()],
                )

            def relu_u_half(nc_, sbuf, md, _extra):
                # uxw tile [P, m_subtiles, 512]: cols 0:256 are u
                nc_.scalar.activation(
                    sbuf[:, :, 0:D],
                    sbuf[:, :, 0:D],
                    mybir.ActivationFunctionType.Relu,
                )

            # uxw = x @ [W_high | W_conv], relu on the u half (replicated
            # on every core — cheaper than an AllGather barrier).
            matmul_tile_kernel(
                tc,
                xT[:],
                Wcat[:],
                uxw[:],
                post_mxn_tile_fn=relu_u_half,
            )

            with (
                tc.tile_pool(name="big_stream", bufs=5) as pool_big,
                tc.tile_pool(name="thin_cache", bufs=9) as pool_cache,
            ):

                def stage(kxm4, kxn_thin, mxn, out_cols=False):
                    # one call per contiguous 512-wide m-block
                    for t in range(NBLK):
                        if out_cols:
                            mxn_t = mxn[:, t * MB : (t + 1) * MB]
                        else:
                            mxn_t = mxn[t * MB : (t + 1) * MB, :]
                        matmul_tile_kernel(
                            tc,
                            kxm4[t] if not out_cols else kxn_thin,
                            kxn_thin if not out_cols else kxm4[t],
                            mxn_t,
                            kxm_pool=pool_big if not out_cols else pool_cache,
                            kxn_pool=pool_cache if not out_cols else pool_big,
                            MAX_K_TILE_SIZE=1024,
                        )

                # v_c = d_inv[rows_c] @ u
                stage(dT4, u, v_c[:])
                allgather(v_c, v)

                # w_c = (aH * lap)[rows_c] @ v
                stage(lapT4, v[:], w_c[:])
                allgather(w_c, w)

                # aggT = (aL*A[rows_c] @ xw).T = xw.T @ AT  (independent of
                # v/w — its matmuls fill the AllGather bubble)
                stage(AT4, xw, aggT[:], out_cols=True)

                # hhT = (d_inv[rows_c] @ w).T = w.T @ dT
                stage(dT4, w[:], hhT[:], out_cols=True)

            # outT = relu(aggT + b') + hhT
            with tc.tile_pool(name="combine", bufs=2) as gp:
                for h in range(2):
                    rows = slice(h * 128, (h + 1) * 128)
                    at_t = gp.tile([128, RPC], f32, name="at")
                    hh_t = gp.tile([128, RPC], f32, name="hh")
                    b_t = gp.tile([128, 1], f32, name="bt")
                    nc.sync.dma_start(at_t[:], aggT[rows, :])
                    nc.sync.dma_start(hh_t[:], hhT[rows, :])
                    nc.sync.dma_start(b_t[:], bp[rows, :])
                    nc.scalar.activation(
                        at_t[:],
                        at_t[:],
                        mybir.ActivationFunctionType.Relu,
                        bias=b_t[:],
                    )
                    nc.vector.tensor_add(at_t[:], at_t[:], hh_t[:])
                    nc.sync.dma_start(outT[rows, :], at_t[:])

    _split_excess_waits(nc, mybir)
    return nc


def _get_program():
    if "nc" not in _PROGRAM_CACHE:
        _PROGRAM_CACHE["nc"] = _build_program()
    return _PROGRAM_CACHE["nc"]


def _tile_big(mat_t):
    """[N, RPC] (k-major) -> [NBLK, 128, N//128, MB] fp16 so that each
    [pi, po-slice, :] kxm/kxn tile DMA is contiguous per partition."""
    m16 = np.asarray(mat_t, dtype=np.float16)
    nblk = RPC // MB
    return np.ascontiguousarray(
        m16.reshape(N // 128, 128, nblk, MB).transpose(2, 1, 0, 3)
    )


def _host_prep(x, edge_index, lap, d_inv, W_high, W_conv, b_conv, aL, aH):
    x = np.asarray(x, dtype=np.float32)
    edge_index = np.asarray(edge_index)
    lap = np.asarray(lap, dtype=np.float32)
    d_inv = np.asarray(d_inv, dtype=np.float32)
    W_high = np.asarray(W_high, dtype=np.float32)
    W_conv = np.asarray(W_conv, dtype=np.float32)
    b_conv = np.asarray(b_conv, dtype=np.float32)
    aL = float(np.asarray(aL).reshape(-1)[0])
    aH = float(np.asarray(aH).reshape(-1)[0])

    n = x.shape[0]
    src = edge_index[0].astype(np.int64)
    dst = edge_index[1].astype(np.int64)
    loops = np.arange(n, dtype=np.int64)
    src_all = np.concatenate([src, loops])
    dst_all = np.concatenate([dst, loops])

    deg = np.bincount(dst_all, minlength=n).astype(np.float32)
    dis = np.where(deg > 0, 1.0 / np.sqrt(np.maximum(deg, 1.0)), 0.0).astype(
        np.float32
    )
    # aL folded into the adjacency (aL*relu(y) == relu(aL*y), aL >= 0)
    norm_all = (dis[src_all] * dis[dst_all] * np.float32(aL)).astype(np.float32)

    AT_full = np.zeros((n, n), dtype=np.float32)  # AT[src, dst]
    np.add.at(AT_full, (src_all, dst_all), norm_all)

    bprime = (np.float32(aL) * b_conv).reshape(D, 1).astype(np.float32)
    xT16 = np.ascontiguousarray(x.T.astype(np.float16))  # [256, N]
    Wcat16 = np.ascontiguousarray(
        np.concatenate([W_high, W_conv], axis=1).astype(np.float16)
    )

    in_maps = []
    for c in range(N_CORES):
        rows = slice(c * RPC, (c + 1) * RPC)
        in_maps.append(
            {
                "xT": xT16,
                "Wcat": Wcat16,
                "bp": bprime,
                "dT4": _tile_big(d_inv[rows, :].T),
                "lapT4": _tile_big(lap[rows, :].T * np.float32(aH)),
                "AT4": _tile_big(AT_full[:, rows]),
            }
        )
    return in_maps


def kernel(
    x,
    edge_index,
    lap,
    d_inv,
    W_high,
    W_conv,
    b_conv,
    aL,
    aH,
    _profile=False,
):
    global LAST_EXEC_NS, LAST_RESULTS
    from concourse.bass_utils import run_bass_kernel_spmd

    if _profile:
        _install_ntff_hook()

    in_maps = _host_prep(
        x, edge_index, lap, d_inv, W_high, W_conv, b_conv, aL, aH
    )
    nc = _get_program()
    res = run_bass_kernel_spmd(
        nc, in_maps, list(range(N_CORES)), trace=bool(_profile)
    )
    LAST_EXEC_NS = res.exec_time_ns
    LAST_RESULTS = res
    out = np.concatenate(
        [res.results[c]["outT"].T for c in range(N_CORES)], axis=0
    )
    return np.ascontiguousarray(out.astype(np.float32))
